# revision 1
# baseline (speedup 1.0000x reference)
"""Trainium2 Bass kernel for the 21x21 correlation (cost volume) module.

Math: out[b, di*21+dj, i, j] = sum_c x1p[b, c, i+di, j+dj] * x2[b, c, i, j]
where x1p is x1 zero-padded by 10 on both spatial dims, di,dj in [0,21).

Strategy (8 NeuronCores, SPMD, no collectives):
  - Shard: batch (4) x W-halves (2). Core k -> (b = k//2, rows i in
    [64*(k%2), 64*(k%2)+64)). x1 shipped with a 10-row halo and +-10
    column padding, zero-filled on the host.
  - On-core: channels C=128 live on the SBUF partition dim (= matmul
    contraction K). For each 8x16 pixel patch, one 128-wide stationary
    operand (the x2 pixels) is multiplied against the streamed 28x36
    window of x1 positions (two matmuls of N=504), producing the
    all-pairs patch product PSUM[pixel, position]. That is copied to
    SBUF (ScalarE + VectorE in parallel) and DMA'd out as a dense
    [128, 1008] block per patch.
  - The band extraction (pixel-relative displacement gather) is a pure
    shear, which no uniform access pattern on the compute engines can
    express; it is done for free on the host with as_strided over the
    gathered [8, 8, 128, 1008] per-core output.
"""
import sys

if "/opt/trn_rl_repo" not in sys.path:
    sys.path.insert(0, "/opt/trn_rl_repo")

import numpy as np
from numpy.lib.stride_tricks import as_strided

import concourse.bass as bass
import concourse.mybir as mybir
import concourse.tile as tile
from concourse import bacc
from concourse.bass_utils import run_bass_kernel_spmd

B, C, W, H = 4, 128, 128, 128
DW = 21          # displacement window (per axis)
PAD = 10
N_CORES = 8
IB, JB = 8, 8            # patch grid per core (8 i-blocks x 8 j-blocks)
PI, PJ = 8, 16           # patch shape (pixels)
RW, QW = PI + DW - 1, PJ + DW - 1    # streamed window 28 x 36
NSTREAM = RW * QW        # 1008
HALO_ROWS = 64 + 2 * PAD     # 84
PADDED_COLS = H + 2 * PAD    # 148

# Matmul input dtype: float32r = full-rate reduced-precision fp32 matmul
# (fp32 storage). Flip to mybir.dt.float32 for exact-but-4x-slower.
MM_DT = mybir.dt.float32r

_CACHE = {}


def _build_program():
    nc = bacc.Bacc("TRN2", target_bir_lowering=False, debug=False,
                   num_devices=N_CORES)
    x1h = nc.dram_tensor("x1h", [C, HALO_ROWS, PADDED_COLS], mybir.dt.float32,
                         kind="ExternalInput")
    # x2 shipped patch-major: [c, ib, jb, pi*pj] so each patch's stationary
    # operand is a single contiguous 128-element free run.
    x2s = nc.dram_tensor("x2s", [C, IB, JB, PI * PJ], MM_DT,
                         kind="ExternalInput")
    outp = nc.dram_tensor("outp", [IB, JB, 128, NSTREAM], mybir.dt.float32,
                          kind="ExternalOutput")

    with tile.TileContext(nc) as tc:
        with (
            tc.tile_pool(name="singles", bufs=1) as singles,
            tc.tile_pool(name="outs", bufs=6) as outs,
            tc.tile_pool(name="repack", bufs=3) as repack,
            tc.tile_pool(name="psum", bufs=4, space="PSUM") as psum,
        ):
            x1_sb = singles.tile([C, HALO_ROWS, PADDED_COLS], mybir.dt.float32)
            x2_sb = singles.tile([C, IB, JB, PI * PJ], MM_DT)
            nc.sync.dma_start(out=x1_sb, in_=x1h[:, :, :])
            nc.sync.dma_start(out=x2_sb, in_=x2s[:, :, :, :])

            for ib in range(IB):
                for jb in range(JB):
                    lhsT = x2_sb[:, ib, jb, :]
                    ps0 = psum.tile([128, 504], mybir.dt.float32, name="ps0")
                    ps1 = psum.tile([128, 504], mybir.dt.float32, name="ps1")
                    # Repack the strided 28x36 x1 window into a contiguous
                    # run so the matmul rhs has a single free dim.
                    rp = repack.tile([128, RW, QW], MM_DT)
                    nc.scalar.copy(out=rp,
                                   in_=x1_sb[:, ib * PI:ib * PI + RW,
                                             jb * PJ:jb * PJ + QW])
                    rpf = rp.rearrange("p a b -> p (a b)")
                    nc.tensor.matmul(ps0, lhsT=lhsT,
                                     rhs=rpf[:, 0:504],
                                     start=True, stop=True)
                    nc.tensor.matmul(ps1, lhsT=lhsT,
                                     rhs=rpf[:, 504:NSTREAM],
                                     start=True, stop=True)
                    ot = outs.tile([128, NSTREAM], mybir.dt.float32)
                    nc.vector.tensor_copy(ot[:, 0:504], ps0)
                    nc.vector.tensor_copy(ot[:, 504:NSTREAM], ps1)
                    nc.sync.dma_start(out=outp[ib, jb], in_=ot)

    nc.finalize()
    return nc


def _shard_inputs(x1, x2):
    in_maps = []
    for k in range(N_CORES):
        b, half = divmod(k, 2)
        i0 = 64 * half
        x2sh = np.ascontiguousarray(
            x2[b][:, i0:i0 + 64, :]
            .reshape(C, IB, PI, JB, PJ)
            .transpose(0, 1, 3, 2, 4)
            .reshape(C, IB, JB, PI * PJ)
        )
        x1sh = np.zeros((C, HALO_ROWS, PADDED_COLS), np.float32)
        rlo, rhi = i0 - PAD, i0 + 64 + PAD
        slo, shi = max(rlo, 0), min(rhi, W)
        x1sh[:, slo - rlo:shi - rlo, PAD:PAD + H] = x1[b][:, slo:shi, :]
        in_maps.append({"x1h": x1sh, "x2s": x2sh})
    return in_maps


def _gather(results):
    out = np.empty((B, DW * DW, W, H), np.float32)
    for k in range(N_CORES):
        b, half = divmod(k, 2)
        i0 = 64 * half
        O = np.ascontiguousarray(results[k]["outp"])  # [8, 8, 128, 1008]
        e = O.itemsize
        s = O.strides
        sv = as_strided(
            O,
            shape=(IB, PI, JB, PJ, DW, DW),
            strides=(s[0], PJ * NSTREAM * e + QW * e, s[1],
                     NSTREAM * e + e, QW * e, e),
        )
        out[b, :, i0:i0 + 64, :] = (
            sv.transpose(4, 5, 0, 1, 2, 3).reshape(DW * DW, 64, H)
        )
    return out


def kernel(x1, x2):
    x1 = np.asarray(x1, dtype=np.float32)
    x2 = np.asarray(x2, dtype=np.float32)
    if "nc" not in _CACHE:
        _CACHE["nc"] = _build_program()
    nc = _CACHE["nc"]
    in_maps = _shard_inputs(x1, x2)
    res = run_bass_kernel_spmd(nc, in_maps, list(range(N_CORES)))
    return _gather(res.results)



# revision 4
# speedup vs baseline: 1.7508x; 1.7508x over previous
"""Trainium2 Bass kernel for the 21x21 correlation (cost volume) module.

Math: out[b, di*21+dj, i, j] = sum_c x1p[b, c, i+di, j+dj] * x2[b, c, i, j]
where x1p is x1 zero-padded by 10 on both spatial dims, di,dj in [0,21).

Strategy (8 NeuronCores, SPMD, no collectives):
  - Shard: batch (4) x W-halves (2). Core k -> (b = k//2, rows i in
    [64*(k%2), 64*(k%2)+64)). Inputs shipped as fp16 (host cast; the
    2e-2 rel-err budget dwarfs fp16 quantization).
  - On-core: channels C=128 on the SBUF partition dim (= matmul K).
    Patches of 16x8 pixels (pi-major partition order p = pi*8+pj); the
    36x28 x1 window is streamed STRAIGHT from the resident x1 tile via
    a strided 3-dim rhs AP (no repack). Two matmuls per patch
    (N=504 = 18x28 window-halves) produce PSUM[pixel, window-pos].
  - PSUM evacuation is split across DVE + Act + GpSimd into a per-band
    [128, 16, 1008] fp16 tile, casting fp32->fp16 in the copy.
  - Output DMA extracts only window rows pi..pi+21 per pi-PAIR of
    partitions (22 rows x 28 cols = 616 of the 1008 per pixel, a 1.40x
    write inflation instead of the dense 2.29x), 8 DMAs per band with
    1232-byte runs. Host de-shears the (di,dj) band with as_strided
    for free and casts back to fp32.
"""
import sys

if "/opt/trn_rl_repo" not in sys.path:
    sys.path.insert(0, "/opt/trn_rl_repo")

import numpy as np
from numpy.lib.stride_tricks import as_strided

import concourse.bass as bass
import concourse.mybir as mybir
import concourse.tile as tile
from concourse import bacc
from concourse.bass_utils import run_bass_kernel_spmd

B, C, W, H = 4, 128, 128, 128
DW = 21          # displacement window (per axis)
PAD = 10
N_CORES = 8
PI, PJ = 16, 8           # patch shape (pixels); partition p = pi*8 + pj
IB, JB = 4, 16           # patch grid per core (4 row-bands x 16 col-patches)
RW, QW = PI + DW - 1, PJ + DW - 1    # streamed window 36 x 28
NSTREAM = RW * QW        # 1008
NPAIR = PI // 2          # 8 pi-pairs per band
ROWS_PAIR = DW + 1       # 22 window rows cover a pi-pair's bands
EPP = ROWS_PAIR * QW     # 616 elements written per pixel
HALO_ROWS = 64 + 2 * PAD     # 84
PADDED_COLS = H + 2 * PAD    # 148

F16 = mybir.dt.float16

_CACHE = {}


def _build_program():
    nc = bacc.Bacc("TRN2", target_bir_lowering=False, debug=False,
                   num_devices=N_CORES)
    x1h = nc.dram_tensor("x1h", [C, HALO_ROWS, PADDED_COLS], F16,
                         kind="ExternalInput")
    # x2 shipped patch-major: [c, ib, jb, p] with p = pi*8 + pj.
    x2s = nc.dram_tensor("x2s", [C, IB, JB, PI * PJ], F16,
                         kind="ExternalInput")
    outp = nc.dram_tensor("outp", [IB, NPAIR, 16, JB, EPP], F16,
                          kind="ExternalOutput")

    with tile.TileContext(nc) as tc:
        with (
            tc.tile_pool(name="singles", bufs=1) as singles,
            tc.tile_pool(name="outs", bufs=2) as outs,
            tc.tile_pool(name="psum", bufs=4, space="PSUM") as psum,
        ):
            x1_sb = singles.tile([C, HALO_ROWS, PADDED_COLS], F16)
            x2_sb = singles.tile([C, IB, JB, PI * PJ], F16)
            nc.sync.dma_start(out=x1_sb, in_=x1h[:, :, :])
            nc.sync.dma_start(out=x2_sb, in_=x2s[:, :, :, :])

            for ib in range(IB):
                ot = outs.tile([128, JB, NSTREAM], F16)
                for jb in range(JB):
                    lhsT = x2_sb[:, ib, jb, :]
                    win = x1_sb[:, ib * PI:ib * PI + RW,
                                jb * PJ:jb * PJ + QW]
                    ps0 = psum.tile([128, 504], mybir.dt.float32, name="ps0")
                    ps1 = psum.tile([128, 504], mybir.dt.float32, name="ps1")
                    nc.tensor.matmul(ps0, lhsT=lhsT, rhs=win[:, 0:18, :],
                                     start=True, stop=True)
                    nc.tensor.matmul(ps1, lhsT=lhsT, rhs=win[:, 18:36, :],
                                     start=True, stop=True)
                    # Evacuate 1008 fp32 PSUM elems -> fp16 on the two
                    # engines that can read PSUM (GpSimd cannot, per the
                    # BIR verifier): DVE takes ps0, Act takes ps1.
                    nc.vector.tensor_copy(ot[:, jb, 0:504], ps0)
                    nc.scalar.copy(out=ot[:, jb, 504:1008], in_=ps1)
                for k in range(NPAIR):
                    # pi-pair {2k, 2k+1} = partitions [16k, 16k+16);
                    # window rows 2k..2k+21 -> elems [56k, 56k+616).
                    nc.sync.dma_start(
                        out=outp[ib, k],
                        in_=ot[16 * k:16 * k + 16, :,
                               56 * k:56 * k + EPP])

    nc.finalize()
    return nc


def _shard_inputs(x1, x2):
    in_maps = []
    for k in range(N_CORES):
        b, half = divmod(k, 2)
        i0 = 64 * half
        x2sh = np.ascontiguousarray(
            x2[b][:, i0:i0 + 64, :]
            .reshape(C, IB, PI, JB, PJ)
            .transpose(0, 1, 3, 2, 4)
            .reshape(C, IB, JB, PI * PJ)
        ).astype(np.float16)
        x1sh = np.zeros((C, HALO_ROWS, PADDED_COLS), np.float16)
        rlo, rhi = i0 - PAD, i0 + 64 + PAD
        slo, shi = max(rlo, 0), min(rhi, W)
        x1sh[:, slo - rlo:shi - rlo, PAD:PAD + H] = \
            x1[b][:, slo:shi, :].astype(np.float16)
        in_maps.append({"x1h": x1sh, "x2s": x2sh})
    return in_maps


def _gather(results):
    out = np.empty((B, DW * DW, W, H), np.float32)
    for k in range(N_CORES):
        b, half = divmod(k, 2)
        i0 = 64 * half
        O = np.ascontiguousarray(results[k]["outp"])  # [4, 8, 16, 16, 616] f16
        e = O.itemsize
        s_ib, s_pair, s_part, s_jb = (np.array(O.strides[:4]) // e)
        # view[ib, pair, pil, pj, jb, di, dj] =
        #   O[ib, pair, pil*8+pj, jb, (pil+di)*28 + pj+dj]
        sv = as_strided(
            O,
            shape=(IB, NPAIR, 2, PJ, JB, DW, DW),
            strides=tuple(np.array(
                [s_ib, s_pair, 8 * s_part + QW, s_part + 1, s_jb, QW, 1]
            ) * e),
        )
        # -> [di, dj, ib, pair, pil, jb, pj] -> [441, 64, 128]
        out[b, :, i0:i0 + 64, :] = (
            sv.transpose(5, 6, 0, 1, 2, 4, 3)
            .reshape(DW * DW, 64, H)
            .astype(np.float32)
        )
    return out


def kernel(x1, x2):
    x1 = np.asarray(x1, dtype=np.float32)
    x2 = np.asarray(x2, dtype=np.float32)
    if "nc" not in _CACHE:
        _CACHE["nc"] = _build_program()
    nc = _CACHE["nc"]
    in_maps = _shard_inputs(x1, x2)
    res = run_bass_kernel_spmd(nc, in_maps, list(range(N_CORES)))
    return _gather(res.results)


# revision 5
# speedup vs baseline: 2.0108x; 1.1485x over previous
"""Trainium2 Bass kernel for the 21x21 correlation (cost volume) module.

Math: out[b, di*21+dj, i, j] = sum_c x1p[b, c, i+di, j+dj] * x2[b, c, i, j]
where x1p is x1 zero-padded by 10 on both spatial dims, di,dj in [0,21).

Strategy (8 NeuronCores, SPMD, no collectives):
  - Shard: batch (4) x W-halves (2). Core k -> (b = k//2, rows i in
    [64*(k%2), 64*(k%2)+64)). Inputs shipped as fp16 (host cast; the
    2e-2 rel-err budget dwarfs fp16 quantization).
  - On-core: channels C=128 on the SBUF partition dim (= matmul K).
    Patches of 16x8 pixels (pi-major partition order p = pi*8+pj); the
    36x28 x1 window is streamed STRAIGHT from the resident x1 tile via
    a strided 3-dim rhs AP (no repack). Two matmuls per patch
    (N=504 = 18x28 window-halves) produce PSUM[pixel, window-pos].
  - PSUM evacuation is split across DVE + Act + GpSimd into a per-band
    [128, 16, 1008] fp16 tile, casting fp32->fp16 in the copy.
  - Output DMA extracts only window rows pi..pi+21 per pi-PAIR of
    partitions (22 rows x 28 cols = 616 of the 1008 per pixel, a 1.40x
    write inflation instead of the dense 2.29x), 8 DMAs per band with
    1232-byte runs. Host de-shears the (di,dj) band with as_strided
    for free and casts back to fp32.
"""
import sys

if "/opt/trn_rl_repo" not in sys.path:
    sys.path.insert(0, "/opt/trn_rl_repo")

import numpy as np
from numpy.lib.stride_tricks import as_strided

import concourse.bass as bass
import concourse.mybir as mybir
import concourse.tile as tile
from concourse import bacc
from concourse.bass_utils import run_bass_kernel_spmd

B, C, W, H = 4, 128, 128, 128
DW = 21          # displacement window (per axis)
PAD = 10
N_CORES = 8
PI, PJ = 16, 8           # patch shape (pixels); partition p = pi*8 + pj
IB, JB = 4, 16           # patch grid per core (4 row-bands x 16 col-patches)
RW, QW = PI + DW - 1, PJ + DW - 1    # streamed window 36 x 28
NSTREAM = RW * QW        # 1008
NPAIR = PI // 2          # 8 pi-pairs per band
ROWS_PAIR = DW + 1       # 22 window rows cover a pi-pair's bands
EPP = ROWS_PAIR * QW     # 616 elements written per pixel
HALO_ROWS = 64 + 2 * PAD     # 84
PADDED_COLS = H + 2 * PAD    # 148

F16 = mybir.dt.float16

_CACHE = {}


def _build_program():
    nc = bacc.Bacc("TRN2", target_bir_lowering=False, debug=False,
                   num_devices=N_CORES)
    x1h = nc.dram_tensor("x1h", [C, HALO_ROWS, PADDED_COLS], F16,
                         kind="ExternalInput")
    # x2 shipped patch-major: [c, ib, jb, p] with p = pi*8 + pj.
    x2s = nc.dram_tensor("x2s", [C, IB, JB, PI * PJ], F16,
                         kind="ExternalInput")
    outp = nc.dram_tensor("outp", [IB, NPAIR, 16, JB, EPP], F16,
                          kind="ExternalOutput")

    with tile.TileContext(nc) as tc:
        with (
            tc.tile_pool(name="singles", bufs=1) as singles,
            tc.tile_pool(name="outs", bufs=2) as outs,
            tc.tile_pool(name="psum", bufs=4, space="PSUM") as psum,
        ):
            x1_sb = singles.tile([C, HALO_ROWS, PADDED_COLS], F16)
            x2_sb = singles.tile([C, IB, JB, PI * PJ], F16)
            # Chunked loads so band 0's matmuls start after ~1/4 of the
            # input traffic instead of all of it.
            nc.sync.dma_start(out=x2_sb[:, 0], in_=x2s[:, 0])
            nc.sync.dma_start(out=x1_sb[:, 0:36], in_=x1h[:, 0:36])
            for ib in range(1, IB):
                r0, r1 = ib * 16 + 20, min(ib * 16 + 36, HALO_ROWS)
                nc.sync.dma_start(out=x1_sb[:, r0:r1], in_=x1h[:, r0:r1])
                nc.sync.dma_start(out=x2_sb[:, ib], in_=x2s[:, ib])

            for ib in range(IB):
                ot = outs.tile([128, JB, NSTREAM], F16)
                for jb in range(JB):
                    lhsT = x2_sb[:, ib, jb, :]
                    win = x1_sb[:, ib * PI:ib * PI + RW,
                                jb * PJ:jb * PJ + QW]
                    ps0 = psum.tile([128, 504], mybir.dt.float32, name="ps0")
                    ps1 = psum.tile([128, 504], mybir.dt.float32, name="ps1")
                    nc.tensor.matmul(ps0, lhsT=lhsT, rhs=win[:, 0:18, :],
                                     start=True, stop=True)
                    nc.tensor.matmul(ps1, lhsT=lhsT, rhs=win[:, 18:36, :],
                                     start=True, stop=True)
                    # Evacuate 1008 fp32 PSUM elems -> fp16 on the two
                    # engines that can read PSUM (GpSimd cannot, per the
                    # BIR verifier): DVE takes ps0, Act takes ps1.
                    nc.vector.tensor_copy(ot[:, jb, 0:504], ps0)
                    nc.scalar.copy(out=ot[:, jb, 504:1008], in_=ps1)
                for k in range(NPAIR):
                    # pi-pair {2k, 2k+1} = partitions [16k, 16k+16);
                    # window rows 2k..2k+21 -> elems [56k, 56k+616).
                    nc.sync.dma_start(
                        out=outp[ib, k],
                        in_=ot[16 * k:16 * k + 16, :,
                               56 * k:56 * k + EPP])

    nc.finalize()
    return nc


def _shard_inputs(x1, x2):
    in_maps = []
    for k in range(N_CORES):
        b, half = divmod(k, 2)
        i0 = 64 * half
        x2sh = np.ascontiguousarray(
            x2[b][:, i0:i0 + 64, :]
            .reshape(C, IB, PI, JB, PJ)
            .transpose(0, 1, 3, 2, 4)
            .reshape(C, IB, JB, PI * PJ)
        ).astype(np.float16)
        x1sh = np.zeros((C, HALO_ROWS, PADDED_COLS), np.float16)
        rlo, rhi = i0 - PAD, i0 + 64 + PAD
        slo, shi = max(rlo, 0), min(rhi, W)
        x1sh[:, slo - rlo:shi - rlo, PAD:PAD + H] = \
            x1[b][:, slo:shi, :].astype(np.float16)
        in_maps.append({"x1h": x1sh, "x2s": x2sh})
    return in_maps


def _gather(results):
    out = np.empty((B, DW * DW, W, H), np.float32)
    for k in range(N_CORES):
        b, half = divmod(k, 2)
        i0 = 64 * half
        O = np.ascontiguousarray(results[k]["outp"])  # [4, 8, 16, 16, 616] f16
        e = O.itemsize
        s_ib, s_pair, s_part, s_jb = (np.array(O.strides[:4]) // e)
        # view[ib, pair, pil, pj, jb, di, dj] =
        #   O[ib, pair, pil*8+pj, jb, (pil+di)*28 + pj+dj]
        sv = as_strided(
            O,
            shape=(IB, NPAIR, 2, PJ, JB, DW, DW),
            strides=tuple(np.array(
                [s_ib, s_pair, 8 * s_part + QW, s_part + 1, s_jb, QW, 1]
            ) * e),
        )
        # -> [di, dj, ib, pair, pil, jb, pj] -> [441, 64, 128]
        out[b, :, i0:i0 + 64, :] = (
            sv.transpose(5, 6, 0, 1, 2, 4, 3)
            .reshape(DW * DW, 64, H)
            .astype(np.float32)
        )
    return out


def kernel(x1, x2):
    x1 = np.asarray(x1, dtype=np.float32)
    x2 = np.asarray(x2, dtype=np.float32)
    if "nc" not in _CACHE:
        _CACHE["nc"] = _build_program()
    nc = _CACHE["nc"]
    in_maps = _shard_inputs(x1, x2)
    res = run_bass_kernel_spmd(nc, in_maps, list(range(N_CORES)))
    return _gather(res.results)


# revision 21
# speedup vs baseline: 2.3876x; 1.1874x over previous
"""Trainium2 Bass kernel for the 21x21 correlation (cost volume) module.

Math: out[b, di*21+dj, i, j] = sum_c x1p[b, c, i+di, j+dj] * x2[b, c, i, j]
where x1p is x1 zero-padded by 10 on both spatial dims, di,dj in [0,21).

Strategy (8 NeuronCores, SPMD, no collectives):
  - Shard: batch (4) x W-halves (2). Core k -> (b = k//2, rows i in
    [64*(k%2), 64*(k%2)+64)). Inputs shipped as fp16 (host cast; the
    2e-2 rel-err budget dwarfs fp16 quantization).
  - On-core: channels C=128 on the SBUF partition dim (= matmul K).
    Patches of 16x8 pixels (pi-major partition order p = pi*8+pj); the
    36x28 x1 window streams STRAIGHT from the resident x1 tile via a
    strided 3-dim rhs AP (no repack). Two matmuls per patch (N=504 =
    18x28 window halves) write one 2-bank PSUM tile at elem offsets
    8 and 512, so each half stays inside a 2KB bank yet the pair is
    contiguous at [8:1016] for a single evacuation copy.
  - Evacuation: one whole-patch copy per engine (DVE takes even
    patches, Act odd ones), fp32 -> fp16 cast in the copy. GpSimd
    cannot read PSUM on TRN2, so these are the only two lanes.
  - Output DMA per (band, pi-pair) extracts only window rows
    2k..2k+21 (22x28 = 616 of 1008 per pixel, 1.40x inflation vs the
    dense 2.29x) with 1232-byte runs. The last band splits each pair
    DMA in jb-halves so shipping starts before the band finishes.
    Host de-shears the (di,dj) band with as_strided for free and
    casts back to fp32.
  - Input DMAs are chunked (first x1 chunk split in column halves,
    first x2 chunk in jb quarters) so the first matmul starts after
    ~2 us of input traffic instead of all 14 us.
"""
import sys

if "/opt/trn_rl_repo" not in sys.path:
    sys.path.insert(0, "/opt/trn_rl_repo")

import numpy as np
from numpy.lib.stride_tricks import as_strided

import concourse.bass as bass
import concourse.mybir as mybir
import concourse.tile as tile
from concourse import bacc
from concourse.bass_utils import run_bass_kernel_spmd

B, C, W, H = 4, 128, 128, 128
DW = 21          # displacement window (per axis)
PAD = 10
N_CORES = 8
PI, PJ = 16, 8           # patch shape (pixels); partition p = pi*8 + pj
IB, JB = 4, 16           # patch grid per core (4 row-bands x 16 col-patches)
RW, QW = PI + DW - 1, PJ + DW - 1    # streamed window 36 x 28
NSTREAM = RW * QW        # 1008
NPAIR = PI // 2          # 8 pi-pairs per band
ROWS_PAIR = DW + 1       # 22 window rows cover a pi-pair
EPP = ROWS_PAIR * QW     # 616 elements written per pixel (pair DMAs)
EPQ = (DW + 3) * QW      # 672: 24 window rows cover a pi-quad
HALO_ROWS = 64 + 2 * PAD     # 84
PADDED_COLS = H + 2 * PAD    # 148

F16 = mybir.dt.float16
F32 = mybir.dt.float32

_CACHE = {}


def _build_program():
    nc = bacc.Bacc("TRN2", target_bir_lowering=False, debug=False,
                   num_devices=N_CORES)
    x1h = nc.dram_tensor("x1h", [C, HALO_ROWS, PADDED_COLS], F16,
                         kind="ExternalInput")
    # x2 shipped patch-major: [c, ib, jb, p] with p = pi*8 + pj.
    x2s = nc.dram_tensor("x2s", [C, IB, JB, PI * PJ], F16,
                         kind="ExternalInput")
    # Bands 0..IB-2 ship as pi-pairs; the last band ships as pi-quads
    # from half-band tiles (fewer, earlier-gated DMAs in the tail).
    outp = nc.dram_tensor("outp", [IB - 1, NPAIR, 16, JB, EPP], F16,
                          kind="ExternalOutput")
    outq = nc.dram_tensor("outq", [2, 4, 32, JB // 2, EPQ], F16,
                          kind="ExternalOutput")

    with tile.TileContext(nc) as tc:
        with (
            tc.tile_pool(name="singles", bufs=1) as singles,
            tc.tile_pool(name="outs", bufs=2) as outs,
            tc.tile_pool(name="psum", bufs=2, space="PSUM") as psum,
        ):
            x1_sb = singles.tile([C, HALO_ROWS, PADDED_COLS], F16)
            x2_sb = singles.tile([C, IB, JB, PI * PJ], F16)
            # Chunked loads, finest pieces first, so band 0's first
            # patches start compute almost immediately.
            nc.sync.dma_start(out=x2_sb[:, 0, 0:2], in_=x2s[:, 0, 0:2])
            nc.sync.dma_start(out=x1_sb[:, 0:18], in_=x1h[:, 0:18])
            nc.sync.dma_start(out=x1_sb[:, 18:36], in_=x1h[:, 18:36])
            nc.sync.dma_start(out=x2_sb[:, 0, 2:16], in_=x2s[:, 0, 2:16])
            for ib in range(1, IB):
                r0, r1 = ib * 16 + 20, min(ib * 16 + 36, HALO_ROWS)
                nc.sync.dma_start(out=x1_sb[:, r0:r1], in_=x1h[:, r0:r1])
                nc.sync.dma_start(out=x2_sb[:, ib], in_=x2s[:, ib])

            def do_patch(ib, jb, ps):
                lhsT = x2_sb[:, ib, jb, :]
                win = x1_sb[:, ib * PI:ib * PI + RW,
                            jb * PJ:jb * PJ + QW]
                nc.tensor.matmul(ps[:, 8:512], lhsT=lhsT,
                                 rhs=win[:, 0:18, :], start=True, stop=True)
                nc.tensor.matmul(ps[:, 512:1016], lhsT=lhsT,
                                 rhs=win[:, 18:36, :], start=True, stop=True)

            def compute(ib, ot, jb_lo, jb_hi):
                for jp in range(jb_lo // 2, jb_hi // 2):
                    jb0, jb1 = 2 * jp, 2 * jp + 1
                    pa = psum.tile([128, 1024], F32, name="pa")
                    pb = psum.tile([128, 1024], F32, name="pb")
                    do_patch(ib, jb0, pa)
                    do_patch(ib, jb1, pb)
                    # One whole-patch evacuation copy per engine (the
                    # only two engines that can read PSUM).
                    nc.vector.tensor_copy(ot[:, jb0 - jb_lo, :],
                                          pa[:, 8:1016])
                    nc.scalar.copy(out=ot[:, jb1 - jb_lo, :],
                                   in_=pb[:, 8:1016])

            # Full-band tiles with pair-granular DMAs for bands 0..IB-2;
            # the last band runs in half-band tiles shipping pi-quads,
            # so its (fewer) DMAs start at the half-band mark instead of
            # all queuing after the final evacuation.
            for ib in range(IB - 1):
                ot = outs.tile([128, JB, NSTREAM], F16)
                compute(ib, ot, 0, JB)
                for k in range(NPAIR):
                    # pi-pair {2k, 2k+1} = partitions [16k, 16k+16);
                    # window rows 2k..2k+21 -> elems [56k, 56k+616).
                    nc.sync.dma_start(
                        out=outp[ib, k],
                        in_=ot[16 * k:16 * k + 16, :,
                               56 * k:56 * k + EPP])
            for h in range(2):
                oth = outs.tile([128, JB // 2, NSTREAM], F16, name="oth")
                compute(IB - 1, oth, h * (JB // 2), (h + 1) * (JB // 2))
                for k in range(4):
                    # pi-quad {4k..4k+3} = partitions [32k, 32k+32);
                    # window rows 4k..4k+23 -> elems [112k, 112k+672).
                    nc.sync.dma_start(
                        out=outq[h, k],
                        in_=oth[32 * k:32 * k + 32, :,
                                112 * k:112 * k + EPQ])

    nc.finalize()
    return nc


def _shard_inputs(x1, x2):
    in_maps = []
    for k in range(N_CORES):
        b, half = divmod(k, 2)
        i0 = 64 * half
        x2sh = np.ascontiguousarray(
            x2[b][:, i0:i0 + 64, :]
            .reshape(C, IB, PI, JB, PJ)
            .transpose(0, 1, 3, 2, 4)
            .reshape(C, IB, JB, PI * PJ)
        ).astype(np.float16)
        x1sh = np.zeros((C, HALO_ROWS, PADDED_COLS), np.float16)
        rlo, rhi = i0 - PAD, i0 + 64 + PAD
        slo, shi = max(rlo, 0), min(rhi, W)
        x1sh[:, slo - rlo:shi - rlo, PAD:PAD + H] = \
            x1[b][:, slo:shi, :].astype(np.float16)
        in_maps.append({"x1h": x1sh, "x2s": x2sh})
    return in_maps


def _gather(results):
    out = np.empty((B, DW * DW, W, H), np.float32)
    for k in range(N_CORES):
        b, half = divmod(k, 2)
        i0 = 64 * half
        # Bands 0..IB-2 from pair staging [IB-1, 8, 16, JB, 616]:
        # O[ib, pair, pil*8+pj, jb, (pil+di)*28 + pj+dj]
        O = np.ascontiguousarray(results[k]["outp"])
        e = O.itemsize
        s_ib, s_pair, s_part, s_jb = (np.array(O.strides[:4]) // e)
        sv = as_strided(
            O,
            shape=(IB - 1, NPAIR, 2, PJ, JB, DW, DW),
            strides=tuple(np.array(
                [s_ib, s_pair, 8 * s_part + QW, s_part + 1, s_jb, QW, 1]
            ) * e),
        )
        out[b, :, i0:i0 + 48, :] = (
            sv.transpose(5, 6, 0, 1, 2, 4, 3)
            .reshape(DW * DW, 48, H)
            .astype(np.float32)
        )
        # Last band from quad staging [2, 4, 32, JB/2, 672]:
        # Q[h, quad, pil*8+pj, jbh, (pil+di)*28 + pj+dj], pi = 4*quad+pil
        Q = np.ascontiguousarray(results[k]["outq"])
        e = Q.itemsize
        q_h, q_quad, q_part, q_jb = (np.array(Q.strides[:4]) // e)
        qv = as_strided(
            Q,
            shape=(2, 4, 4, PJ, JB // 2, DW, DW),
            strides=tuple(np.array(
                [q_h, q_quad, 8 * q_part + QW, q_part + 1, q_jb, QW, 1]
            ) * e),
        )
        # axes -> [di, dj, quad, pil, h, jbh, pj] -> [441, 16, 128]
        out[b, :, i0 + 48:i0 + 64, :] = (
            qv.transpose(5, 6, 1, 2, 0, 4, 3)
            .reshape(DW * DW, 16, H)
            .astype(np.float32)
        )
    return out


def kernel(x1, x2):
    x1 = np.asarray(x1, dtype=np.float32)
    x2 = np.asarray(x2, dtype=np.float32)
    if "nc" not in _CACHE:
        _CACHE["nc"] = _build_program()
    nc = _CACHE["nc"]
    in_maps = _shard_inputs(x1, x2)
    res = run_bass_kernel_spmd(nc, in_maps, list(range(N_CORES)))
    return _gather(res.results)


# revision 22
# speedup vs baseline: 2.4085x; 1.0088x over previous
"""Trainium2 Bass kernel for the 21x21 correlation (cost volume) module.

Math: out[b, di*21+dj, i, j] = sum_c x1p[b, c, i+di, j+dj] * x2[b, c, i, j]
where x1p is x1 zero-padded by 10 on both spatial dims, di,dj in [0,21).

Strategy (8 NeuronCores, SPMD, no collectives):
  - Shard: batch (4) x W-halves (2). Core k -> (b = k//2, rows i in
    [64*(k%2), 64*(k%2)+64)). Inputs shipped as fp16 (host cast; the
    2e-2 rel-err budget dwarfs fp16 quantization).
  - On-core: channels C=128 on the SBUF partition dim (= matmul K).
    Patches of 16x8 pixels (pi-major partition order p = pi*8+pj); the
    36x28 x1 window streams STRAIGHT from the resident x1 tile via a
    strided 3-dim rhs AP (no repack). Two matmuls per patch (N=504 =
    18x28 window halves) write one 2-bank PSUM tile at elem offsets
    8 and 512, so each half stays inside a 2KB bank yet the pair is
    contiguous at [8:1016] for a single evacuation copy.
  - Evacuation: one whole-patch copy per engine (DVE takes even
    patches, Act odd ones), fp32 -> fp16 cast in the copy. GpSimd
    cannot read PSUM on TRN2, so these are the only two lanes.
  - Output DMA per (band, pi-pair) extracts only window rows
    2k..2k+21 (22x28 = 616 of 1008 per pixel, 1.40x inflation vs the
    dense 2.29x) with 1232-byte runs. The last band splits each pair
    DMA in jb-halves so shipping starts before the band finishes.
    Host de-shears the (di,dj) band with as_strided for free and
    casts back to fp32.
  - Input DMAs are chunked (first x1 chunk split in column halves,
    first x2 chunk in jb quarters) so the first matmul starts after
    ~2 us of input traffic instead of all 14 us.
"""
import sys

if "/opt/trn_rl_repo" not in sys.path:
    sys.path.insert(0, "/opt/trn_rl_repo")

import numpy as np
from numpy.lib.stride_tricks import as_strided

import concourse.bass as bass
import concourse.mybir as mybir
import concourse.tile as tile
from concourse import bacc
from concourse.bass_utils import run_bass_kernel_spmd

B, C, W, H = 4, 128, 128, 128
DW = 21          # displacement window (per axis)
PAD = 10
N_CORES = 8
PI, PJ = 16, 8           # patch shape (pixels); partition p = pi*8 + pj
IB, JB = 4, 16           # patch grid per core (4 row-bands x 16 col-patches)
RW, QW = PI + DW - 1, PJ + DW - 1    # streamed window 36 x 28
NSTREAM = RW * QW        # 1008
NPAIR = PI // 2          # 8 pi-pairs per band
ROWS_PAIR = DW + 1       # 22 window rows cover a pi-pair
EPP = ROWS_PAIR * QW     # 616 elements written per pixel (pair DMAs)
EPQ = (DW + 3) * QW      # 672: 24 window rows cover a pi-quad
HALO_ROWS = 64 + 2 * PAD     # 84
PADDED_COLS = H + 2 * PAD    # 148

F16 = mybir.dt.float16
F32 = mybir.dt.float32

_CACHE = {}


def _build_program():
    nc = bacc.Bacc("TRN2", target_bir_lowering=False, debug=False,
                   num_devices=N_CORES)
    x1h = nc.dram_tensor("x1h", [C, HALO_ROWS, PADDED_COLS], F16,
                         kind="ExternalInput")
    # x2 shipped patch-major: [c, ib, jb, p] with p = pi*8 + pj.
    x2s = nc.dram_tensor("x2s", [C, IB, JB, PI * PJ], F16,
                         kind="ExternalInput")
    # Bands 0..IB-2 ship as pi-pairs; the last band ships as pi-quads
    # from half-band tiles (fewer, earlier-gated DMAs in the tail).
    outp = nc.dram_tensor("outp", [IB - 1, NPAIR, 16, JB, EPP], F16,
                          kind="ExternalOutput")
    outq = nc.dram_tensor("outq", [2, 4, 32, JB // 2, EPQ], F16,
                          kind="ExternalOutput")

    with tile.TileContext(nc) as tc:
        with (
            tc.tile_pool(name="singles", bufs=1) as singles,
            tc.tile_pool(name="outs", bufs=3) as outs,
            tc.tile_pool(name="psum", bufs=2, space="PSUM") as psum,
        ):
            x1_sb = singles.tile([C, HALO_ROWS, PADDED_COLS], F16)
            x2_sb = singles.tile([C, IB, JB, PI * PJ], F16)
            # Chunked loads, finest pieces first, so band 0's first
            # patches start compute almost immediately.
            nc.sync.dma_start(out=x2_sb[:, 0, 0:2], in_=x2s[:, 0, 0:2])
            nc.sync.dma_start(out=x1_sb[:, 0:18], in_=x1h[:, 0:18])
            nc.sync.dma_start(out=x1_sb[:, 18:36], in_=x1h[:, 18:36])
            nc.sync.dma_start(out=x2_sb[:, 0, 2:16], in_=x2s[:, 0, 2:16])
            for ib in range(1, IB):
                r0, r1 = ib * 16 + 20, min(ib * 16 + 36, HALO_ROWS)
                nc.sync.dma_start(out=x1_sb[:, r0:r1], in_=x1h[:, r0:r1])
                nc.sync.dma_start(out=x2_sb[:, ib], in_=x2s[:, ib])

            def do_patch(ib, jb, ps):
                lhsT = x2_sb[:, ib, jb, :]
                win = x1_sb[:, ib * PI:ib * PI + RW,
                            jb * PJ:jb * PJ + QW]
                nc.tensor.matmul(ps[:, 8:512], lhsT=lhsT,
                                 rhs=win[:, 0:18, :], start=True, stop=True)
                nc.tensor.matmul(ps[:, 512:1016], lhsT=lhsT,
                                 rhs=win[:, 18:36, :], start=True, stop=True)

            def compute(ib, ot, jb_lo, jb_hi):
                for jp in range(jb_lo // 2, jb_hi // 2):
                    jb0, jb1 = 2 * jp, 2 * jp + 1
                    pa = psum.tile([128, 1024], F32, name="pa")
                    pb = psum.tile([128, 1024], F32, name="pb")
                    do_patch(ib, jb0, pa)
                    do_patch(ib, jb1, pb)
                    # One whole-patch evacuation copy per engine (the
                    # only two engines that can read PSUM).
                    nc.vector.tensor_copy(ot[:, jb0 - jb_lo, :],
                                          pa[:, 8:1016])
                    nc.scalar.copy(out=ot[:, jb1 - jb_lo, :],
                                   in_=pb[:, 8:1016])

            # Full-band tiles with pair-granular DMAs for bands 0..IB-2;
            # the last band runs in half-band tiles shipping pi-quads,
            # so its (fewer) DMAs start at the half-band mark instead of
            # all queuing after the final evacuation.
            for ib in range(IB - 1):
                ot = outs.tile([128, JB, NSTREAM], F16)
                compute(ib, ot, 0, JB)
                for k in range(NPAIR):
                    # pi-pair {2k, 2k+1} = partitions [16k, 16k+16);
                    # window rows 2k..2k+21 -> elems [56k, 56k+616).
                    nc.sync.dma_start(
                        out=outp[ib, k],
                        in_=ot[16 * k:16 * k + 16, :,
                               56 * k:56 * k + EPP])
            for h in range(2):
                oth = outs.tile([128, JB // 2, NSTREAM], F16, name="oth")
                compute(IB - 1, oth, h * (JB // 2), (h + 1) * (JB // 2))
                for k in range(4):
                    # pi-quad {4k..4k+3} = partitions [32k, 32k+32);
                    # window rows 4k..4k+23 -> elems [112k, 112k+672).
                    nc.sync.dma_start(
                        out=outq[h, k],
                        in_=oth[32 * k:32 * k + 32, :,
                                112 * k:112 * k + EPQ])

    nc.finalize()
    return nc


def _shard_inputs(x1, x2):
    in_maps = []
    for k in range(N_CORES):
        b, half = divmod(k, 2)
        i0 = 64 * half
        x2sh = np.ascontiguousarray(
            x2[b][:, i0:i0 + 64, :]
            .reshape(C, IB, PI, JB, PJ)
            .transpose(0, 1, 3, 2, 4)
            .reshape(C, IB, JB, PI * PJ)
        ).astype(np.float16)
        x1sh = np.zeros((C, HALO_ROWS, PADDED_COLS), np.float16)
        rlo, rhi = i0 - PAD, i0 + 64 + PAD
        slo, shi = max(rlo, 0), min(rhi, W)
        x1sh[:, slo - rlo:shi - rlo, PAD:PAD + H] = \
            x1[b][:, slo:shi, :].astype(np.float16)
        in_maps.append({"x1h": x1sh, "x2s": x2sh})
    return in_maps


def _gather(results):
    out = np.empty((B, DW * DW, W, H), np.float32)
    for k in range(N_CORES):
        b, half = divmod(k, 2)
        i0 = 64 * half
        # Bands 0..IB-2 from pair staging [IB-1, 8, 16, JB, 616]:
        # O[ib, pair, pil*8+pj, jb, (pil+di)*28 + pj+dj]
        O = np.ascontiguousarray(results[k]["outp"])
        e = O.itemsize
        s_ib, s_pair, s_part, s_jb = (np.array(O.strides[:4]) // e)
        sv = as_strided(
            O,
            shape=(IB - 1, NPAIR, 2, PJ, JB, DW, DW),
            strides=tuple(np.array(
                [s_ib, s_pair, 8 * s_part + QW, s_part + 1, s_jb, QW, 1]
            ) * e),
        )
        out[b, :, i0:i0 + 48, :] = (
            sv.transpose(5, 6, 0, 1, 2, 4, 3)
            .reshape(DW * DW, 48, H)
            .astype(np.float32)
        )
        # Last band from quad staging [2, 4, 32, JB/2, 672]:
        # Q[h, quad, pil*8+pj, jbh, (pil+di)*28 + pj+dj], pi = 4*quad+pil
        Q = np.ascontiguousarray(results[k]["outq"])
        e = Q.itemsize
        q_h, q_quad, q_part, q_jb = (np.array(Q.strides[:4]) // e)
        qv = as_strided(
            Q,
            shape=(2, 4, 4, PJ, JB // 2, DW, DW),
            strides=tuple(np.array(
                [q_h, q_quad, 8 * q_part + QW, q_part + 1, q_jb, QW, 1]
            ) * e),
        )
        # axes -> [di, dj, quad, pil, h, jbh, pj] -> [441, 16, 128]
        out[b, :, i0 + 48:i0 + 64, :] = (
            qv.transpose(5, 6, 1, 2, 0, 4, 3)
            .reshape(DW * DW, 16, H)
            .astype(np.float32)
        )
    return out


def kernel(x1, x2):
    x1 = np.asarray(x1, dtype=np.float32)
    x2 = np.asarray(x2, dtype=np.float32)
    if "nc" not in _CACHE:
        _CACHE["nc"] = _build_program()
    nc = _CACHE["nc"]
    in_maps = _shard_inputs(x1, x2)
    res = run_bass_kernel_spmd(nc, in_maps, list(range(N_CORES)))
    return _gather(res.results)
